# revision 1
# baseline (speedup 1.0000x reference)
"""Multi-head attention Trainium2 kernel (8 NeuronCores).

Problem: B=4, N=2048, D=64, H=12 multi-head attention with per-head QKV
projections, softmax attention, concat + output projection (fp32).

Sharding: 8 cores = 4 batches x 2 head-groups (6 heads each; the
"tensor parallel over heads" option from the sharding hint). Each core
computes a partial output projection for its batch; the host sums the two
head-group partials per batch (the reduce after the output projection),
transposes, and adds the output bias.

Device algorithm (per core; data fp32; matmuls float32r = full-rate
single-pass fp32, ~1.5e-4 matmul rounding; P/V in bf16):
  - x arrives host-pre-transposed as xT [64, 2048] (host layout prep,
    like the weight stacks; saves 16 PE transposes + copies on device)
  - QT/KT per head-pair packed on partitions ([128, 2048]: even head rows
    0:64, odd head rows 64:128; the odd half is placed via SBUF->SBUF DMA
    because col-packed fp32r matmuls and partition-shifted DVE ops are
    rejected by this toolchain)
  - V natural [k, e] for all 6 heads at once (lhsT = xT chunk, rhs = Wv
    stacked [64, 384]), stored interleaved as [V_h | 1] (65-wide groups);
    the ones column makes the AV matmul emit [OT ; softmax denominator]
  - scores transposed ST[k, q] = K @ Q.T via row-packed matmul pairs
    (tile_position row groups 0/64), grouped 3 x 512 q-slots per 3-bank
    PSUM tile; exp on ScalarE straight out of PSUM with the 1/sqrt(D)
    scale fused (no max-subtraction needed: |scores| <~ 6 in fp32)
  - AV + output projection run on the raw (unnormalized) OT: the per-q
    softmax normalization commutes with the output projection and is
    applied afterwards on GpSimd (outacc += po * recip), with the
    reciprocal broadcast across partitions by a K=1 ones-outer-product
    matmul -- so neither the PE nor the DVE ever blocks on it
  - software pipeline: iteration 0 streams transposes/projections into
    the first scores so exp starts within ~5us; afterwards AV(i) runs
    while ScalarE exps iteration i+1; pair p+1 projections are
    interleaved into pair p's iterations

The walrus build here accepts only one sync-wait per instruction, so a
BIR post-pass splits Tile's multi-wait instructions onto NoOps (see
_split_excess_waits). Timing (reps-slope on HW): ~350 us/core end-to-end;
cost-model sim predicts 251 us with ScalarE (the exp stream, 188 us busy)
and the PE (190 us) as the near-balanced bottlenecks.
"""
import os
import sys

sys.path.insert(0, "/opt/trn_rl_repo")

# The kernel needs jax's axon (NeuronCore) backend. If the environment
# pinned JAX_PLATFORMS to something that excludes it (e.g. "cpu" for
# running the reference) and jax hasn't been imported yet, undo that.
_jp = os.environ.get("JAX_PLATFORMS")
if _jp and "axon" not in _jp and "jax" not in sys.modules:
    os.environ["JAX_PLATFORMS"] = ""

import numpy as np

import concourse.bass as bass
import concourse.tile as tile
from concourse import mybir

B, N, D, H = 4, 2048, 64, 12
NH = 6            # heads per core
NPAIR = 3         # head pairs per core
NKC = N // 128    # 16 k-chunks
QW = 512          # q tile width
NQC = N // QW     # 4 q-chunks
F32 = mybir.dt.float32
F32R = mybir.dt.float32r
BF16 = mybir.dt.bfloat16

# ---------------------------------------------------------------------------
# This walrus build accepts only ONE sync wait command per instruction
# ("Too many sync wait commands" codegen error otherwise), while Tile emits
# instructions with several semaphore waits. Split excess waits onto NoOp
# instructions inserted just before the offender (same engine, so engine
# program order makes them execute first) by rewriting the BIR JSON on its
# way into the backend compiler.
# ---------------------------------------------------------------------------
_MAXW = 1


def _split_excess_waits(bir: dict) -> dict:
    counter = [0]

    def fix_block(b):
        insts = b.get("instructions")
        if insts:
            out = []
            for ins in insts:
                si = ins.get("sync_info")
                w = (si or {}).get("on_wait") or []
                if len(w) > _MAXW:
                    for k in range(0, len(w) - _MAXW, _MAXW):
                        counter[0] += 1
                        out.append({
                            "name": f"WSPL-{counter[0]}",
                            "opcode": "NoOp",
                            "engine": ins["engine"],
                            "ins": [],
                            "outs": [],
                            "debug": ins.get("debug", 0),
                            "sync_info": {
                                "on_wait": w[k:k + _MAXW],
                                "on_update": [],
                            },
                        })
                    si["on_wait"] = w[len(w) - _MAXW:]
                out.append(ins)
            b["instructions"] = out
        for sb in b.get("blocks", []) or []:
            fix_block(sb)

    for fn in bir.get("functions", []):
        for blk in fn.get("blocks", []):
            fix_block(blk)
    return bir


def _install_wait_split_hook():
    import json as _json

    import concourse.bass2jax as _b2j
    import concourse.bass_utils as _bu

    if getattr(_bu, "_wait_split_installed", False):
        return
    _orig = _bu.compile_bir_kernel

    def _cbk(bir_json, tmpdir, neff_name="file.neff"):
        if isinstance(bir_json, str):
            bir_json = bir_json.encode()
        d = _json.loads(bir_json)
        d = _split_excess_waits(d)
        return _orig(_json.dumps(d).encode(), tmpdir, neff_name=neff_name)

    _bu.compile_bir_kernel = _cbk
    _b2j.compile_bir_kernel = _cbk
    _bu._wait_split_installed = True


_install_wait_split_hook()


def build_nc(reps=1):
    nc = bass.Bass("TRN2", target_bir_lowering=False, debug=False)

    xt_d = nc.dram_tensor("xt", [D, N], F32R, kind="ExternalInput")
    wq_d = nc.dram_tensor("wq", [D, NH * D], F32R, kind="ExternalInput")
    wk_d = nc.dram_tensor("wk", [D, NH * D], F32R, kind="ExternalInput")
    wv_d = nc.dram_tensor("wv", [D, NH * D], F32R, kind="ExternalInput")
    bq_d = nc.dram_tensor("bq6", [D, NH], F32, kind="ExternalInput")
    bk_d = nc.dram_tensor("bk6", [D, NH], F32, kind="ExternalInput")
    bvb_d = nc.dram_tensor("bvb", [128, NH * D], F32, kind="ExternalInput")
    wo_d = nc.dram_tensor("wo", [D, NH * D], F32R, kind="ExternalInput")
    out_d = nc.dram_tensor("out_t", [D, N], F32, kind="ExternalOutput")

    with tile.TileContext(nc) as tc:
        with (
            tc.tile_pool(name="singles", bufs=1) as singles,
            tc.tile_pool(name="ptmp", bufs=3) as ptmp,
            tc.tile_pool(name="pP", bufs=23) as pP,
        ):
            xT = singles.tile([D, N], F32R)
            wq_sb = singles.tile([D, NH * D], F32R)
            wk_sb = singles.tile([D, NH * D], F32R)
            wv_sb = singles.tile([D, NH * D], F32R)
            wo_sb = singles.tile([D, NH * D], F32R)
            bq_sb = singles.tile([D, NH], F32)
            bk_sb = singles.tile([D, NH], F32)
            bvb_sb = singles.tile([128, NH * D], F32)
            ones_f32 = singles.tile([128, NKC * NH], F32)
            ones_bc = singles.tile([128, D], F32R)
            QKDT = BF16 if os.environ.get("K_BF16_QK") else F32R
            UNPACKED = bool(os.environ.get("K_UNPACKED"))
            _qkrows = 64 if UNPACKED else 128
            _nqk = NH if UNPACKED else NPAIR
            QT2 = [singles.tile([_qkrows, N], QKDT, name=f"QT2_{i}",
                                tag=f"QT2_{i}") for i in range(_nqk)]
            KT2 = [singles.tile([_qkrows, N], QKDT, name=f"KT2_{i}",
                                tag=f"KT2_{i}") for i in range(_nqk)]
            Vn = singles.tile([128, NKC, NH, D + 1], BF16)
            outacc = singles.tile([D, N], F32)

            nc.sync.dma_start(wq_sb[:], wq_d[:])
            nc.sync.dma_start(wk_sb[:], wk_d[:])
            nc.sync.dma_start(bq_sb[:], bq_d[:])
            nc.sync.dma_start(bk_sb[:], bk_d[:])

            # preload the exp table set during the input DMAs so the first
            # real exp doesn't pay the ~2.7us ACT_TABLE_LOAD
            nc.vector.memset(ones_f32[:, 0:1], 0.0)
            nc.scalar.activation(
                ones_f32[:, 0:1], ones_f32[:, 0:1],
                mybir.ActivationFunctionType.Exp, scale=1.0,
            )

            # ones columns of the [V_h | 1] groups (fused softmax denominator)
            nc.vector.memset(ones_f32[:], 1.0)
            nc.vector.tensor_copy(
                ones_bc[:], ones_f32[:, 0:D]
            )
            nc.vector.tensor_copy(
                Vn[:, :, :, D:D + 1],
                ones_f32[:].rearrange("p (c h) -> p c h", c=NKC)[:, :, :, None],
            )

            for _rep in range(reps):
                # PSUM budget (8 banks): pscore 2 x [128,1536] = 6 banks,
                # psmall 2 x [128,512] = 2 banks. Everything small
                # (transposes, projections, V, AV, outproj) shares psmall.
                with (
                    tc.tile_pool(name="pscore", bufs=2, space="PSUM") as pscore,
                    tc.tile_pool(name="psmall", bufs=2, space="PSUM") as psmall,
                ):
                    SLOTS = 2 * NKC  # 32 matmul outputs of QW cols per iter

                    def emit_tr(c_lo, c_hi):
                        # x arrives pre-transposed from the host (layout
                        # prep, like the weight stacks) -- just DMA it in
                        nc.sync.dma_start(
                            xT[:, c_lo * 128:c_hi * 128],
                            xt_d[:, c_lo * 128:c_hi * 128],
                        )

                    def emit_proj(p, qc):
                        # Q/K projections for head pair p, q-chunk qc; odd
                        # head rows go to partitions 64:128 via SBUF DMA
                        # (col-packed matmuls / partition-shifted DVE ops are
                        # unavailable for fp32r)
                        qs = slice(qc * QW, (qc + 1) * QW)
                        for (w_sb, b_sb, dst) in (
                            (wq_sb, bq_sb, QT2),
                            (wk_sb, bk_sb, KT2),
                        ):
                            for hi in range(2):
                                hh = 2 * p + hi
                                ps = psmall.tile([128, QW], F32, tag="sm")
                                nc.tensor.matmul(
                                    ps[0:D, :],
                                    w_sb[:, hh * D:(hh + 1) * D],
                                    xT[:, qs],
                                    start=True, stop=True,
                                )
                                if UNPACKED:
                                    nc.vector.tensor_scalar_add(
                                        dst[hh][:, qs], ps[0:D, :],
                                        b_sb[:, hh:hh + 1],
                                    )
                                elif hi == 0:
                                    nc.vector.tensor_scalar_add(
                                        dst[p][0:D, qs], ps[0:D, :],
                                        b_sb[:, hh:hh + 1],
                                    )
                                else:
                                    tmp = ptmp.tile([D, QW], QKDT, tag="ptmp")
                                    nc.vector.tensor_scalar_add(
                                        tmp[:], ps[0:D, :], b_sb[:, hh:hh + 1],
                                    )
                                    nc.sync.dma_start(dst[p][D:128, qs], tmp[:])

                    def emit_v(c_lo, c_hi):
                        # V natural (+bias) for all heads, one matmul/chunk
                        for c in range(c_lo, c_hi):
                            pv = psmall.tile([128, QW], F32, tag="sm")
                            nc.tensor.matmul(
                                pv[:, 0:NH * D],
                                xT[:, c * 128:(c + 1) * 128],
                                wv_sb[:],
                                start=True, stop=True,
                            )
                            nc.vector.tensor_tensor(
                                Vn[:, c, :, 0:D],
                                pv[:, 0:NH * D].rearrange(
                                    "p (h e) -> p h e", h=NH),
                                bvb_sb[:].rearrange("p (h e) -> p h e", h=NH),
                                mybir.AluOpType.add,
                            )

                    class ScoreEmitter:
                        """Row-packed scores matmuls + exp, grouped three
                        QW-slots per 3-bank psum tile for wide ACT reads."""

                        def __init__(self, p, qc):
                            self.p, self.qc = p, qc
                            self.qs = slice(qc * QW, (qc + 1) * QW)
                            self.ptiles = []
                            self.stile = None

                        def emit(self, c_lo, c_hi):
                            for c in range(c_lo, c_hi):
                                ks = slice(c * 128, (c + 1) * 128)
                                for hi in range(2):
                                    s = 2 * c + hi
                                    pos = s % 3
                                    if pos == 0:
                                        self.width = min(3, SLOTS - s) * QW
                                        self.stile = pscore.tile(
                                            [128, 1536], F32, tag="sc",
                                            name="sc")
                                        ptile = pP.tile(
                                            [128, 1536], BF16, tag="pexp",
                                            name="pexp")
                                        self.ptiles.append(ptile)
                                    if UNPACKED:
                                        hh = 2 * self.p + hi
                                        nc.tensor.matmul(
                                            self.stile[:, pos * QW:(pos + 1) * QW],
                                            KT2[hh][:, ks],
                                            QT2[hh][:, self.qs],
                                            start=True, stop=True,
                                            tile_position=(0, 0),
                                        )
                                    else:
                                        base = 0 if hi == 0 else 64
                                        nc.tensor.matmul(
                                            self.stile[:, pos * QW:(pos + 1) * QW],
                                            KT2[self.p][base:base + 64, ks],
                                            QT2[self.p][base:base + 64, self.qs],
                                            start=True, stop=True,
                                            tile_position=(base, 0),
                                        )
                                    if pos == self.width // QW - 1 \
                                            or s == SLOTS - 1:
                                        nc.scalar.activation(
                                            self.ptiles[-1][:, 0:(pos + 1) * QW],
                                            self.stile[:, 0:(pos + 1) * QW],
                                            mybir.ActivationFunctionType.Exp,
                                            scale=1.0 / 8.0,
                                        )

                    def emit_av(p, qc, ptiles):
                        # AV + output projection on the raw (unnormalized)
                        # OT: the per-q softmax normalization commutes with
                        # the output projection and is applied afterwards on
                        # GpSimd (outacc += po * recip), so neither the PE
                        # nor the DVE ever waits on the DRAM-round-trip
                        # reciprocal broadcast.
                        qs = slice(qc * QW, (qc + 1) * QW)

                        def pslice(c, hi):
                            s = 2 * c + hi
                            return ptiles[s // 3][
                                :, (s % 3) * QW:(s % 3 + 1) * QW]

                        for hi in range(2):
                            hh = 2 * p + hi
                            pav_t = psmall.tile([128, QW], F32, tag="sm")
                            for c in range(NKC):
                                nc.tensor.matmul(
                                    pav_t[0:D + 1, :],
                                    Vn[:, c, hh, :],
                                    pslice(c, hi),
                                    start=(c == 0), stop=(c == NKC - 1),
                                )
                            _no_norm = bool(os.environ.get("K_NO_NORM"))
                            rec = ptmp.tile([128, QW], F32R, tag="rec")
                            with nc.allow_low_precision(
                                reason="f32r recip feeds K=1 broadcast matmul"
                            ):
                                nc.vector.reciprocal(
                                    rec[D:D + 1, :], pav_t[D:D + 1, :]
                                )
                            ot_raw = ptmp.tile([D, QW], F32R, tag="otraw")
                            nc.vector.tensor_copy(ot_raw[:], pav_t[0:D, :])
                            po = psmall.tile([128, QW], F32, tag="sm")
                            nc.tensor.matmul(
                                po[0:D, :],
                                wo_sb[:, hh * D:(hh + 1) * D],
                                ot_raw[:],
                                start=True, stop=True,
                            )
                            if not _no_norm:
                                # broadcast recip across partitions with a
                                # K=1 outer-product matmul (ones x recip)
                                pbc = psmall.tile([128, QW], F32, tag="sm")
                                nc.tensor.matmul(
                                    pbc[0:D, :],
                                    ones_bc[D:D + 1, :],
                                    rec[D:D + 1, :],
                                    start=True, stop=True,
                                )
                                rb = ptmp.tile([D, QW], F32, tag="rb")
                                nc.vector.tensor_copy(rb[:], pbc[0:D, :])
                            po_sb = ptmp.tile([D, QW], F32, tag="posb")
                            nc.vector.tensor_copy(po_sb[:], po[0:D, :])
                            tsc = ptmp.tile([D, QW], F32, tag="tsc")
                            if _no_norm:
                                nc.gpsimd.tensor_copy(tsc[:], po_sb[:])
                            else:
                                nc.gpsimd.tensor_mul(tsc[:], po_sb[:], rb[:])
                            if p == 0 and hi == 0:
                                nc.gpsimd.tensor_copy(outacc[:, qs], tsc[:])
                            else:
                                nc.gpsimd.tensor_add(
                                    outacc[:, qs], outacc[:, qs], tsc[:]
                                )
                            if p == NPAIR - 1 and hi == 1:
                                nc.sync.dma_start(
                                    out_d[:, qs], outacc[:, qs]
                                )

                    # Iteration 0 streams the prologue: transposes and
                    # pair-0 projections feed the first scores k-chunks as
                    # soon as their xT columns exist, so exp starts within a
                    # few us of kernel start. V runs after, overlapping the
                    # first exp stream.
                    NIT = NPAIR * NQC
                    se = ScoreEmitter(0, 0)
                    for qc in range(NQC):
                        emit_tr(4 * qc, 4 * qc + 4)
                        emit_proj(0, qc)
                        se.emit(4 * qc, 4 * qc + 4)
                        if qc == 0:
                            # V/out weights are first needed at emit_v /
                            # emit_av(0); keep their DMAs off the critical
                            # front queue
                            nc.sync.dma_start(wv_sb[:], wv_d[:])
                            nc.sync.dma_start(wo_sb[:], wo_d[:])
                            nc.sync.dma_start(bvb_sb[:], bvb_d[:])
                    emit_v(0, NKC)
                    prev = (0, 0, se.ptiles)

                    # steady state: AV for iteration i runs while ScalarE
                    # exps iteration i+1's scores
                    for it in range(1, NIT + 1):
                        if it < NIT:
                            p, qc = divmod(it, NQC)
                            itp = it % NQC
                            if p + 1 < NPAIR and itp >= 1:
                                emit_proj(p + 1, itp - 1)
                                if itp == NQC - 1:
                                    emit_proj(p + 1, itp)
                            if it == NIT - 1 and not os.environ.get("K_XT_ACT"):
                                # last iteration: AV first so the only PE
                                # work left after the final exp is its own AV
                                emit_av(*prev)
                                se = ScoreEmitter(p, qc)
                                se.emit(0, NKC)
                            else:
                                se = ScoreEmitter(p, qc)
                                se.emit(0, NKC)
                                emit_av(*prev)
                            prev = (p, qc, se.ptiles)
                        else:
                            emit_av(*prev)

    return nc


_NC_CACHE = {}


def _get_nc(reps=1):
    if reps not in _NC_CACHE:
        _NC_CACHE[reps] = build_nc(reps)
    return _NC_CACHE[reps]


def prep_in_maps(x, Wq, Wk, Wv, bq, bk, bv, Wo, bo):
    x = np.asarray(x, dtype=np.float32)
    Wq = np.asarray(Wq, dtype=np.float32)
    Wk = np.asarray(Wk, dtype=np.float32)
    Wv = np.asarray(Wv, dtype=np.float32)
    bq = np.asarray(bq, dtype=np.float32)
    bk = np.asarray(bk, dtype=np.float32)
    bv = np.asarray(bv, dtype=np.float32)
    Wo = np.asarray(Wo, dtype=np.float32)

    in_maps = []
    for core in range(8):
        b, g = core // 2, core % 2
        hs = slice(g * NH, (g + 1) * NH)
        wq = np.ascontiguousarray(Wq[hs].transpose(1, 0, 2).reshape(D, NH * D))
        wk = np.ascontiguousarray(Wk[hs].transpose(1, 0, 2).reshape(D, NH * D))
        wv = np.ascontiguousarray(Wv[hs].transpose(1, 0, 2).reshape(D, NH * D))
        wo = np.ascontiguousarray(
            Wo[g * NH * D:(g + 1) * NH * D].reshape(NH, D, D)
            .transpose(1, 0, 2).reshape(D, NH * D)
        )
        bvb = np.ascontiguousarray(
            np.broadcast_to(bv[hs].reshape(1, NH * D), (128, NH * D))
        )
        in_maps.append({
            "xt": np.ascontiguousarray(x[b].T),
            "wq": wq, "wk": wk, "wv": wv,
            "bq6": np.ascontiguousarray(bq[hs].T),
            "bk6": np.ascontiguousarray(bk[hs].T),
            "bvb": bvb, "wo": wo,
        })
    return in_maps


def kernel(x, Wq, Wk, Wv, bq, bk, bv, Wo, bo, _trace=False, _reps=1):
    from concourse.bass_utils import run_bass_kernel_spmd

    bo = np.asarray(bo, dtype=np.float32)
    nc = _get_nc(_reps)
    in_maps = prep_in_maps(x, Wq, Wk, Wv, bq, bk, bv, Wo, bo)

    res = run_bass_kernel_spmd(
        nc, in_maps, core_ids=list(range(8)), trace=_trace
    )

    out = np.empty((B, N, D), dtype=np.float32)
    for b in range(B):
        part = res.results[2 * b]["out_t"] + res.results[2 * b + 1]["out_t"]
        out[b] = part.T + bo[None, :]

    if _trace:
        return out, res
    return out

